# revision 11
# baseline (speedup 1.0000x reference)
"""Trainium2 Bass kernel for the 2-layer heterogeneous GCN encoder.

Strategy (8 NeuronCores, SPMD):
  - Shard each relation's edges by dst-node owner: core k owns user rows
    [k*12500,(k+1)*12500) and item rows [k*6250,(k+1)*6250).
  - Aggregate raw features first (segment_sum(x[src]*norm, dst) per
    512-row dst window via a one-hot S matmul), transform per window
    afterwards: out = agg @ W + b.
  - Feature rows fetched with dma_gather (SWDGE custom op): up to 512
    rows per instruction, round-robined across 2 SWDGE queues (two Q7
    descriptor-gen cores run in parallel; measured ~3ns/row vs ~8.4ns
    single-queue and ~72x less instruction overhead than per-tile
    indirect DMA).
  - dma_gather takes int16 indices, so each relation's edges are grouped
    by (dst window, 32K src chunk); indices are chunk-relative.
  - x_user/x_item are uploaded bf16 and SHARDED (1/8 per core), cast to
    fp32 and AllGathered device-side into full HBM tables -- avoids
    uploading the full tables 8x.
  - Layer-1 outputs AllGathered fp32; layer 2 reads the gathered tables.
  - Outputs written bf16 (host casts back to fp32).

Self-contained: hardcodes problem shapes; host does only index-side prep
(degrees/norms from int32 edge lists, sharding, sorting, packing).
"""

import os
import sys

sys.path.insert(0, "/opt/trn_rl_repo")

import numpy as np
import ml_dtypes

import concourse.bass as bass
import concourse.bacc as bacc
import concourse.mybir as mybir
import concourse.tile as tile
from concourse.bass_utils import run_bass_kernel_spmd
from concourse.library_config import mlp

P = 128
WIN = 512  # dst rows per aggregation window (one PSUM bank)
CHUNK = 32768  # max rows addressable by one int16-indexed gather
GMAX = 4  # max tiles (of 128 rows) per dma_gather op -> <=512 rows
NCORES = 8
NQUEUES = 2
SCRATCH = 65536  # SWDGE ring: 4096 descriptors per queue per direction
F32 = mybir.dt.float32
F16 = mybir.dt.float16
BF16 = mybir.dt.bfloat16
I16 = mybir.dt.int16

CFG = dict(N_U=100000, N_I=50000, E=1600000, D=128)

# relation -> (src table, dst type)
RELS = {
    "follows": ("user", "user"),
    "rates": ("user", "item"),
    "rev": ("item", "user"),
}


def _cdiv(a, b):
    return (a + b - 1) // b


class RelSched:
    """Harmonized (across cores) tile schedule for one relation."""

    __slots__ = ("nwin", "nchunk", "T", "t0", "Ttot", "ops", "chunk_rows")

    def __init__(self, nwin, nchunk, T, chunk_rows):
        self.nwin, self.nchunk = nwin, nchunk
        self.T = T  # [nwin, nchunk] tiles per group
        self.chunk_rows = chunk_rows  # rows per chunk of the src table
        t0 = np.zeros((nwin, nchunk), np.int64)
        flat = T.reshape(-1)
        t0.reshape(-1)[1:] = np.cumsum(flat)[:-1]
        self.t0 = t0
        self.Ttot = int(flat.sum())
        # gather ops per window: list of (chunk, t_start, g_tiles)
        self.ops = []
        for w in range(nwin):
            lst = []
            for c in range(nchunk):
                t, rem = int(t0[w, c]), int(T[w, c])
                while rem > 0:
                    g = min(GMAX, rem)
                    lst.append((c, t, g))
                    t += g
                    rem -= g
            self.ops.append(lst)


def prep_relation(src, dst, n_src, n_dst, ncores=NCORES):
    """Shard edges by dst owner, group by (dst window, src chunk), pack
    per-core streams. Returns (RelSched, per-core (gidx, colw, norm))."""
    shard = n_dst // ncores
    nwin = _cdiv(shard, WIN)
    nchunk = _cdiv(n_src, CHUNK)
    chunk_rows = [min(CHUNK, n_src - c * CHUNK) for c in range(nchunk)]

    ones = np.ones_like(src, dtype=np.float64)
    deg_s = np.bincount(src, weights=ones, minlength=n_src)
    deg_d = np.bincount(dst, weights=ones, minlength=n_dst)
    inv_s = np.where(deg_s > 0, 1.0 / np.sqrt(deg_s), 0.0)
    inv_d = np.where(deg_d > 0, 1.0 / np.sqrt(deg_d), 0.0)
    norm = (inv_s[src] * inv_d[dst]).astype(np.float32)

    owner = dst // shard
    dloc = dst - owner * shard
    w = dloc // WIN
    col = dloc - w * WIN
    chunk = src // CHUNK
    srcrel = (src - chunk * CHUNK).astype(np.int64)

    key = (owner * nwin + w) * nchunk + chunk
    order = np.argsort(key, kind="stable")
    key_s = key[order]
    counts = np.bincount(key_s, minlength=ncores * nwin * nchunk).reshape(
        ncores, nwin, nchunk
    )
    T = _cdiv(counts.max(axis=0), P)  # [nwin, nchunk]
    sched = RelSched(nwin, nchunk, T, chunk_rows)

    # position of each (sorted) edge within its (owner,w,chunk) group
    grp_start = np.zeros(ncores * nwin * nchunk + 1, np.int64)
    grp_start[1:] = np.cumsum(counts.reshape(-1))
    j = np.arange(len(src)) - grp_start[key_s]
    # global token index within the core's stream
    t0_flat = sched.t0.reshape(-1)  # [nwin*nchunk]
    wc_key = key_s % (nwin * nchunk)
    g_tok = t0_flat[wc_key] * P + j

    src_o, col_o, norm_o, own_o = (
        srcrel[order],
        col[order],
        norm[order],
        owner[order],
    )
    per_core = []
    ntok = sched.Ttot * P
    for k in range(ncores):
        sel = own_o == k
        g_k = g_tok[sel]
        gidx = np.zeros((16, ntok // 16), np.int16)
        colw = np.zeros((P, sched.Ttot), np.float16)
        nrmw = np.zeros((P, sched.Ttot), np.float16)
        gidx[g_k % 16, g_k // 16] = src_o[sel].astype(np.int16)
        colw[g_k % P, g_k // P] = col_o[sel].astype(np.float16)
        nrmw[g_k % P, g_k // P] = norm_o[sel].astype(np.float16)
        per_core.append((gidx, colw, nrmw))
    return sched, per_core


def build_program(cfg, scheds):
    N_U, N_I, D = cfg["N_U"], cfg["N_I"], cfg["D"]
    SU, SI = N_U // NCORES, N_I // NCORES

    ABL_NOAG = os.environ.get("ABL_NOAG") == "1"
    ABL_L1ONLY = os.environ.get("ABL_L1ONLY") == "1"
    ABL_NOGATHER = os.environ.get("ABL_NOGATHER") == "1"
    ABL_NOS = os.environ.get("ABL_NOS") == "1"

    nc = bacc.Bacc(
        "TRN2",
        target_bir_lowering=False,
        dynamic_dma_scratch_size=SCRATCH,
        num_swdge_queues=NQUEUES,
    )

    xu_in = nc.dram_tensor("xu_shard", [SU, D], BF16, kind="ExternalInput")
    xi_in = nc.dram_tensor("xi_shard", [SI, D], BF16, kind="ExternalInput")
    Ws = {
        n: nc.dram_tensor(n, [D, D], F32, kind="ExternalInput")
        for n in ["W1_follows", "W1_rates", "W1_rev", "W2_follows", "W2_rates", "W2_rev"]
    }
    bs = {
        n: nc.dram_tensor(n, [D], F32, kind="ExternalInput")
        for n in ["b1_follows", "b1_rates", "b1_rev", "b2_follows", "b2_rates", "b2_rev"]
    }
    iota_in = nc.dram_tensor("iota512", [P, WIN], F32, kind="ExternalInput")
    ident_in = nc.dram_tensor("ident", [P, P], F32, kind="ExternalInput")
    streams = {}
    for r, sched in scheds.items():
        streams[r] = dict(
            gidx=nc.dram_tensor(f"gidx_{r}", [16, sched.Ttot * 8], I16, kind="ExternalInput"),
            colw=nc.dram_tensor(f"colw_{r}", [P, sched.Ttot], F16, kind="ExternalInput"),
            norm=nc.dram_tensor(f"norm_{r}", [P, sched.Ttot], F16, kind="ExternalInput"),
        )
    out_user = nc.dram_tensor("out_user", [SU, D], BF16, kind="ExternalOutput")
    out_item = nc.dram_tensor("out_item", [SI, D], BF16, kind="ExternalOutput")

    # library preamble: ensure Q7 mlp ucode (dma_gather) is loaded before
    # any tile-scheduled gather executes
    with nc.Block() as blk:
        @blk.gpsimd
        def _(gpsimd):
            gpsimd.load_library(mlp)

    with tile.TileContext(nc) as tc:
        with (
            tc.tile_pool(name="const", bufs=1) as cp,
            tc.tile_pool(name="gsl", bufs=10) as gp,
            tc.tile_pool(name="Sp", bufs=8) as sp,
            tc.tile_pool(name="swp", bufs=4) as swp,
            tc.tile_pool(name="gwp", bufs=4) as gwp,
            tc.tile_pool(name="agg", bufs=6) as aggp,
            tc.tile_pool(name="outp", bufs=6) as outp,
            tc.tile_pool(name="ps", bufs=4, space="PSUM") as pp,
            tc.tile_pool(name="psh", bufs=2, space="PSUM") as ph_pool,
            tc.tile_pool(name="pstr", bufs=2, space="PSUM") as ptr,
            tc.tile_pool(name="dram", bufs=1, space="DRAM") as dp,
        ):
            # ---- constants ----
            iota_t = cp.tile([P, WIN], F32, tag="iota")
            nc.sync.dma_start(iota_t[:], iota_in[:])
            ident_t = cp.tile([P, P], F32, tag="ident")
            nc.sync.dma_start(ident_t[:], ident_in[:])
            W_t = {}
            for n, Wd in Ws.items():
                W_t[n] = cp.tile([P, P], F32, tag=f"W_{n}", name=f"W_{n}")
                nc.sync.dma_start(W_t[n][:], Wd[:])
            b_t = {}
            for n, b in bs.items():
                b_t[n] = cp.tile([P, 1], F32, tag=f"b_{n}", name=f"bt_{n}")
                nc.sync.dma_start(b_t[n][:], b[:].unsqueeze(1))
            b1uv = cp.tile([P, 1], F32, tag="b1uv")
            nc.vector.tensor_tensor(
                out=b1uv[:], in0=b_t["b1_follows"][:], in1=b_t["b1_rev"][:],
                op=mybir.AluOpType.add,
            )
            nc.vector.tensor_scalar_mul(b1uv[:], b1uv[:], 0.5)
            b2uv = cp.tile([P, 1], F32, tag="b2uv")
            nc.vector.tensor_tensor(
                out=b2uv[:], in0=b_t["b2_follows"][:], in1=b_t["b2_rev"][:],
                op=mybir.AluOpType.add,
            )
            nc.vector.tensor_scalar_mul(b2uv[:], b2uv[:], 0.5)

            # ---- streams resident in SBUF (gidx streamed per window) ----
            st = {}
            for r, sched in scheds.items():
                colw = cp.tile([P, sched.Ttot], F16, tag=f"colw_{r}", name=f"colwt_{r}")
                nc.sync.dma_start(colw[:], streams[r]["colw"][:])
                nrm = cp.tile([P, sched.Ttot], F16, tag=f"norm_{r}", name=f"normt_{r}")
                nc.sync.dma_start(nrm[:], streams[r]["norm"][:])
                st[r] = dict(colw=colw, norm=nrm)

            # ---- DRAM tables ----
            xu_f32 = dp.tile([SU, D], F32, tag="xu_f32")
            xi_f32 = dp.tile([SI, D], F32, tag="xi_f32")
            xu_full = dp.tile([N_U, D], F32, tag="xu_full", addr_space="Shared")
            xi_full = dp.tile([N_I, D], F32, tag="xi_full", addr_space="Shared")
            u_slice = dp.tile([SU, D], F32, tag="u_slice")
            it_slice = dp.tile([SI, D], F32, tag="it_slice")
            u_full = dp.tile([N_U, D], F32, tag="u_full", addr_space="Shared")
            it_full = dp.tile([N_I, D], F32, tag="it_full", addr_space="Shared")

            # upcast input shards (sync DMA + ACT copy: SWDGE queues are
            # reserved for gathers -- tile locks DMASW sem lanes per queue)
            def upcast(dst_ap, src, rows):
                for b in range(_cdiv(rows, P)):
                    r0, r1 = b * P, min((b + 1) * P, rows)
                    tb = outp.tile([P, P], BF16, tag="castb")
                    nc.sync.dma_start(tb[: r1 - r0, :], src[r0:r1, :])
                    tf = outp.tile([P, P], F32, tag="castf")
                    nc.scalar.activation(
                        out=tf[: r1 - r0, :], in_=tb[: r1 - r0, :],
                        func=mybir.ActivationFunctionType.Copy,
                    )
                    nc.sync.dma_start(dst_ap[r0:r1, :], tf[: r1 - r0, :])

            upcast(xu_f32[:], xu_in.ap(), SU)
            upcast(xi_f32[:], xi_in.ap(), SI)
            grp = [list(range(NCORES))]
            if not ABL_NOAG:
                nc.gpsimd.collective_compute(
                    "AllGather", mybir.AluOpType.bypass, replica_groups=grp,
                    ins=[xu_f32[:]], outs=[xu_full[:]],
                )
                nc.gpsimd.collective_compute(
                    "AllGather", mybir.AluOpType.bypass, replica_groups=grp,
                    ins=[xi_f32[:]], outs=[xi_full[:]],
                )

            qctr = [0]

            def agg_window(rel, w, table_ap):
                """PSUM tile [128 feat, WIN] = sum_e x[src_e] (x) one-hot."""
                sched = scheds[rel]
                ops = sched.ops[w]
                total = sum(g for (_, _, g) in ops)
                psum = pp.tile([P, WIN], F32, tag="aggps")
                if total == 0:
                    nc.vector.memset(psum[:], 0.0)
                    return psum
                # fp32 per-window casts of the fp16 colw/norm streams
                # (DVE is_equal requires an fp32 scalar operand)
                t_w0 = int(sched.t0[w, 0])
                T_win = int(sched.T[w].sum())
                colw32 = swp.tile([P, T_win], F32, tag="colw32")
                nc.scalar.activation(
                    out=colw32[:], in_=st[rel]["colw"][:, t_w0 : t_w0 + T_win],
                    func=mybir.ActivationFunctionType.Copy,
                )
                norm32 = swp.tile([P, T_win], F32, tag="norm32")
                nc.scalar.activation(
                    out=norm32[:], in_=st[rel]["norm"][:, t_w0 : t_w0 + T_win],
                    func=mybir.ActivationFunctionType.Copy,
                )
                # window slice of the wrapped gather indices, replicated
                # into all 8 partition groups via a broadcast AP
                gw = gwp.tile([P, T_win * 8], I16, tag="gw")
                gsrc = streams[rel]["gidx"][:, t_w0 * 8 : (t_w0 + T_win) * 8]
                nc.sync.dma_start(gw[:], gsrc.unsqueeze(0).broadcast_to([8, 16, T_win * 8]))
                done = 0
                for (c, t_start, g) in ops:
                    if ABL_NOGATHER:
                        slab = None
                    else:
                        slab = gp.tile([P, GMAX, P], F32, tag="gsl")
                        nidx = g * P
                        c0 = (t_start - t_w0) * 8
                        nc.gpsimd.dma_gather(
                            slab[:, :g, :],
                            table_ap[c * CHUNK : c * CHUNK + sched.chunk_rows[c], :],
                            gw[:, c0 : c0 + nidx // 16],
                            nidx,
                            nidx,
                            P,
                            queue_num=qctr[0] % NQUEUES,
                        )
                        qctr[0] += 1
                    for j in range(g):
                        t = t_start + j - t_w0
                        if ABL_NOS:
                            S = iota_t
                        else:
                            S = sp.tile([P, WIN], F32, tag="S")
                            nc.vector.tensor_scalar(
                                out=S[:],
                                in0=iota_t[:],
                                scalar1=colw32[:, t : t + 1],
                                scalar2=norm32[:, t : t + 1],
                                op0=mybir.AluOpType.is_equal,
                                op1=mybir.AluOpType.mult,
                            )
                        lhs = ident_t if ABL_NOGATHER else slab[:, j, :]
                        nc.tensor.matmul(
                            out=psum[:],
                            lhsT=lhs,
                            rhs=S[:],
                            start=(done == 0),
                            stop=(done == total - 1),
                        )
                        done += 1
                return psum

            def write_windows(h_sb, w, nrows, dst_ap, out_dt):
                for blk_i in range(_cdiv(nrows, P)):
                    r0, r1 = blk_i * P, min((blk_i + 1) * P, nrows)
                    ptile = ptr.tile([P, P], F32, tag="ptr")
                    nc.tensor.transpose(
                        out=ptile[: r1 - r0, :],
                        in_=h_sb[:, r0:r1],
                        identity=ident_t[:],
                    )
                    ob = outp.tile([P, P], out_dt, tag="ob")
                    nc.scalar.activation(
                        out=ob[: r1 - r0, :], in_=ptile[: r1 - r0, :],
                        func=mybir.ActivationFunctionType.Copy,
                    )
                    nc.sync.dma_start(
                        dst_ap[w * WIN + r0 : w * WIN + r1, :], ob[: r1 - r0, :]
                    )

            def user_layer(l, table_u, table_i, dst_ap, shard_rows, relu, out_dt):
                Wf = W_t[f"W{l}_follows"]
                Wv = W_t[f"W{l}_rev"]
                bias = b1uv if l == 1 else b2uv
                for w in range(_cdiv(shard_rows, WIN)):
                    nrows = min(WIN, shard_rows - w * WIN)
                    psF = agg_window("follows", w, table_u)
                    aggF = aggp.tile([P, WIN], F32, tag="aggF")
                    nc.scalar.activation(
                        out=aggF[:], in_=psF[:], func=mybir.ActivationFunctionType.Copy
                    )
                    psV = agg_window("rev", w, table_i)
                    aggV = aggp.tile([P, WIN], F32, tag="aggV")
                    nc.scalar.activation(
                        out=aggV[:], in_=psV[:], func=mybir.ActivationFunctionType.Copy
                    )
                    hps = ph_pool.tile([P, WIN], F32, tag="hps")
                    nc.tensor.matmul(out=hps[:], lhsT=Wf[:], rhs=aggF[:], start=True, stop=False)
                    nc.tensor.matmul(out=hps[:], lhsT=Wv[:], rhs=aggV[:], start=False, stop=True)
                    h_sb = aggp.tile([P, WIN], F32, tag="h_sb")
                    if relu:
                        nc.scalar.activation(
                            out=h_sb[:], in_=hps[:],
                            func=mybir.ActivationFunctionType.Relu,
                            bias=bias[:], scale=0.5,
                        )
                    else:
                        nc.vector.tensor_scalar(
                            out=h_sb[:], in0=hps[:],
                            scalar1=0.5, scalar2=bias[:],
                            op0=mybir.AluOpType.mult, op1=mybir.AluOpType.add,
                        )
                    write_windows(h_sb, w, nrows, dst_ap, out_dt)

            def item_layer(l, table_u, dst_ap, shard_rows, relu, out_dt):
                Wr = W_t[f"W{l}_rates"]
                bias = b_t[f"b{l}_rates"]
                for w in range(_cdiv(shard_rows, WIN)):
                    nrows = min(WIN, shard_rows - w * WIN)
                    psR = agg_window("rates", w, table_u)
                    aggR = aggp.tile([P, WIN], F32, tag="aggR")
                    nc.scalar.activation(
                        out=aggR[:], in_=psR[:], func=mybir.ActivationFunctionType.Copy
                    )
                    hps = ph_pool.tile([P, WIN], F32, tag="hps")
                    nc.tensor.matmul(out=hps[:], lhsT=Wr[:], rhs=aggR[:], start=True, stop=True)
                    h_sb = aggp.tile([P, WIN], F32, tag="h_sb")
                    if relu:
                        nc.scalar.activation(
                            out=h_sb[:], in_=hps[:],
                            func=mybir.ActivationFunctionType.Relu,
                            bias=bias[:], scale=1.0,
                        )
                    else:
                        nc.vector.tensor_scalar(
                            out=h_sb[:], in0=hps[:],
                            scalar1=1.0, scalar2=bias[:],
                            op0=mybir.AluOpType.mult, op1=mybir.AluOpType.add,
                        )
                    write_windows(h_sb, w, nrows, dst_ap, out_dt)

            # ---- layer 1 ----
            user_layer(1, xu_full[:], xi_full[:], u_slice[:], SU, relu=True, out_dt=F32)
            if not ABL_NOAG:
                nc.gpsimd.collective_compute(
                    "AllGather", mybir.AluOpType.bypass, replica_groups=grp,
                    ins=[u_slice[:]], outs=[u_full[:]],
                )
            item_layer(1, xu_full[:], it_slice[:], SI, relu=True, out_dt=F32)
            if not ABL_NOAG:
                nc.gpsimd.collective_compute(
                    "AllGather", mybir.AluOpType.bypass, replica_groups=grp,
                    ins=[it_slice[:]], outs=[it_full[:]],
                )
            # ---- layer 2 (rates first: only needs u_full) ----
            if not ABL_L1ONLY:
                item_layer(2, u_full[:], out_item.ap(), SI, relu=False, out_dt=BF16)
                user_layer(2, u_full[:], it_full[:], out_user.ap(), SU, relu=False, out_dt=BF16)

    nc.compile()
    return nc


def prepare(inputs):
    """Host-side prep + program build. Returns (nc, in_maps)."""
    cfg = dict(CFG)
    N_U = inputs["x_user"].shape[0]
    N_I = inputs["x_item"].shape[0]
    cfg.update(N_U=N_U, N_I=N_I, E=len(inputs["follows_src"]))
    SU, SI = N_U // NCORES, N_I // NCORES

    rel_edges = {
        "follows": (inputs["follows_src"], inputs["follows_dst"], N_U, N_U),
        "rates": (inputs["rates_src"], inputs["rates_dst"], N_U, N_I),
        "rev": (inputs["rev_src"], inputs["rev_dst"], N_I, N_U),
    }
    scheds, packs = {}, {}
    for r, (s, d, ns, nd) in rel_edges.items():
        sched, per_core = prep_relation(
            np.asarray(s).astype(np.int64), np.asarray(d).astype(np.int64), ns, nd
        )
        scheds[r] = sched
        packs[r] = per_core

    nc = build_program(cfg, scheds)

    iota512 = np.broadcast_to(np.arange(WIN, dtype=np.float32), (P, WIN)).copy()
    ident = np.eye(P, dtype=np.float32)
    xu = np.asarray(inputs["x_user"]).astype(ml_dtypes.bfloat16)
    xi = np.asarray(inputs["x_item"]).astype(ml_dtypes.bfloat16)
    common = {
        n: np.asarray(inputs[n])
        for n in [
            "W1_follows", "W1_rates", "W1_rev", "W2_follows", "W2_rates", "W2_rev",
            "b1_follows", "b1_rates", "b1_rev", "b2_follows", "b2_rates", "b2_rev",
        ]
    }
    in_maps = []
    for k in range(NCORES):
        m = dict(common, iota512=iota512, ident=ident)
        m["xu_shard"] = xu[k * SU : (k + 1) * SU]
        m["xi_shard"] = xi[k * SI : (k + 1) * SI]
        for r in rel_edges:
            gidx, colw, nrmw = packs[r][k]
            m[f"gidx_{r}"] = gidx
            m[f"colw_{r}"] = colw
            m[f"norm_{r}"] = nrmw
        in_maps.append(m)
    return nc, in_maps


def assemble(results):
    u2 = np.concatenate([results[k]["out_user"] for k in range(NCORES)], axis=0)
    i2 = np.concatenate([results[k]["out_item"] for k in range(NCORES)], axis=0)
    return np.concatenate([u2, i2], axis=0).astype(np.float32)


def kernel(**inputs):
    nc, in_maps = prepare(inputs)
    res = run_bass_kernel_spmd(nc, in_maps, list(range(NCORES)))
    return assemble(res.results)


if __name__ == "__main__":
    pass


# revision 13
# speedup vs baseline: 1.4812x; 1.4812x over previous
"""Trainium2 Bass kernel for the 2-layer heterogeneous GCN encoder.

Strategy (8 NeuronCores, SPMD):
  - Shard each relation's edges by dst-node owner: core k owns user rows
    [k*12500,(k+1)*12500) and item rows [k*6250,(k+1)*6250).
  - Aggregate raw features first (segment_sum(x[src]*norm, dst) per
    512-row dst window via a one-hot S matmul), transform per window
    afterwards: out = agg @ W + b.
  - Feature rows fetched with dma_gather (SWDGE custom op): up to 512
    rows per instruction, round-robined across 2 SWDGE queues (two Q7
    descriptor-gen cores run in parallel; measured ~3ns/row vs ~8.4ns
    single-queue and ~72x less instruction overhead than per-tile
    indirect DMA).
  - dma_gather takes int16 indices, so each relation's edges are grouped
    by (dst window, 32K src chunk); indices are chunk-relative.
  - x_user/x_item are uploaded bf16 and SHARDED (1/8 per core), cast to
    fp32 and AllGathered device-side into full HBM tables -- avoids
    uploading the full tables 8x.
  - Layer-1 outputs AllGathered fp32; layer 2 reads the gathered tables.
  - Outputs written bf16 (host casts back to fp32).

Self-contained: hardcodes problem shapes; host does only index-side prep
(degrees/norms from int32 edge lists, sharding, sorting, packing).
"""

import os
import sys

sys.path.insert(0, "/opt/trn_rl_repo")

import numpy as np
import ml_dtypes

import concourse.bass as bass
import concourse.bacc as bacc
import concourse.mybir as mybir
import concourse.tile as tile
from concourse.bass_utils import run_bass_kernel_spmd
from concourse.library_config import mlp

P = 128
WIN = 512  # dst rows per aggregation window (one PSUM bank)
CHUNK = 32768  # max rows addressable by one int16-indexed gather
GMAX = 4  # max tiles (of 128 rows) per dma_gather op -> <=512 rows
NCORES = 8
NQUEUES = 2
SCRATCH = 65536  # SWDGE ring: 4096 descriptors per queue per direction
F32 = mybir.dt.float32
F16 = mybir.dt.float16
BF16 = mybir.dt.bfloat16
I16 = mybir.dt.int16

CFG = dict(N_U=100000, N_I=50000, E=1600000, D=128)

# relation -> (src table, dst type)
RELS = {
    "follows": ("user", "user"),
    "rates": ("user", "item"),
    "rev": ("item", "user"),
}


def _cdiv(a, b):
    return (a + b - 1) // b


class RelSched:
    """Harmonized (across cores) tile schedule for one relation."""

    __slots__ = ("nwin", "nchunk", "T", "t0", "Ttot", "ops", "chunk_rows")

    def __init__(self, nwin, nchunk, T, chunk_rows):
        self.nwin, self.nchunk = nwin, nchunk
        self.T = T  # [nwin, nchunk] tiles per group
        self.chunk_rows = chunk_rows  # rows per chunk of the src table
        t0 = np.zeros((nwin, nchunk), np.int64)
        flat = T.reshape(-1)
        t0.reshape(-1)[1:] = np.cumsum(flat)[:-1]
        self.t0 = t0
        self.Ttot = int(flat.sum())
        # gather ops per window: list of (chunk, t_start, g_tiles)
        self.ops = []
        for w in range(nwin):
            lst = []
            for c in range(nchunk):
                t, rem = int(t0[w, c]), int(T[w, c])
                while rem > 0:
                    g = min(GMAX, rem)
                    lst.append((c, t, g))
                    t += g
                    rem -= g
            self.ops.append(lst)


def prep_relation(src, dst, n_src, n_dst, ncores=NCORES):
    """Shard edges by dst owner, group by (dst window, src chunk), pack
    per-core streams. Returns (RelSched, per-core (gidx, colw, norm))."""
    shard = n_dst // ncores
    nwin = _cdiv(shard, WIN)
    nchunk = _cdiv(n_src, CHUNK)
    chunk_rows = [min(CHUNK, n_src - c * CHUNK) for c in range(nchunk)]

    ones = np.ones_like(src, dtype=np.float64)
    deg_s = np.bincount(src, weights=ones, minlength=n_src)
    deg_d = np.bincount(dst, weights=ones, minlength=n_dst)
    inv_s = np.where(deg_s > 0, 1.0 / np.sqrt(deg_s), 0.0)
    inv_d = np.where(deg_d > 0, 1.0 / np.sqrt(deg_d), 0.0)
    norm = (inv_s[src] * inv_d[dst]).astype(np.float32)

    owner = dst // shard
    dloc = dst - owner * shard
    w = dloc // WIN
    col = dloc - w * WIN
    chunk = src // CHUNK
    srcrel = (src - chunk * CHUNK).astype(np.int64)

    key = (owner * nwin + w) * nchunk + chunk
    order = np.argsort(key, kind="stable")
    key_s = key[order]
    counts = np.bincount(key_s, minlength=ncores * nwin * nchunk).reshape(
        ncores, nwin, nchunk
    )
    T = _cdiv(counts.max(axis=0), P)  # [nwin, nchunk]
    sched = RelSched(nwin, nchunk, T, chunk_rows)

    # position of each (sorted) edge within its (owner,w,chunk) group
    grp_start = np.zeros(ncores * nwin * nchunk + 1, np.int64)
    grp_start[1:] = np.cumsum(counts.reshape(-1))
    j = np.arange(len(src)) - grp_start[key_s]
    # global token index within the core's stream
    t0_flat = sched.t0.reshape(-1)  # [nwin*nchunk]
    wc_key = key_s % (nwin * nchunk)
    g_tok = t0_flat[wc_key] * P + j

    src_o, col_o, norm_o, own_o = (
        srcrel[order],
        col[order],
        norm[order],
        owner[order],
    )
    per_core = []
    ntok = sched.Ttot * P
    for k in range(ncores):
        sel = own_o == k
        g_k = g_tok[sel]
        gidx = np.zeros((16, ntok // 16), np.int16)
        colw = np.zeros((P, sched.Ttot), np.float16)
        nrmw = np.zeros((P, sched.Ttot), np.float16)
        gidx[g_k % 16, g_k // 16] = src_o[sel].astype(np.int16)
        colw[g_k % P, g_k // P] = col_o[sel].astype(np.float16)
        nrmw[g_k % P, g_k // P] = norm_o[sel].astype(np.float16)
        per_core.append((gidx, colw, nrmw))
    return sched, per_core


def build_program(cfg, scheds):
    N_U, N_I, D = cfg["N_U"], cfg["N_I"], cfg["D"]
    SU, SI = N_U // NCORES, N_I // NCORES

    ABL_NOAG = os.environ.get("ABL_NOAG") == "1"
    ABL_L1ONLY = os.environ.get("ABL_L1ONLY") == "1"
    ABL_NOGATHER = os.environ.get("ABL_NOGATHER") == "1"
    ABL_NOS = os.environ.get("ABL_NOS") == "1"

    nc = bacc.Bacc(
        "TRN2",
        target_bir_lowering=False,
        dynamic_dma_scratch_size=SCRATCH,
        num_swdge_queues=NQUEUES,
    )

    xu_in = nc.dram_tensor("xu_shard", [SU, D], BF16, kind="ExternalInput")
    xi_in = nc.dram_tensor("xi_shard", [SI, D], BF16, kind="ExternalInput")
    Ws = {
        n: nc.dram_tensor(n, [D, D], F32, kind="ExternalInput")
        for n in ["W1_follows", "W1_rates", "W1_rev", "W2_follows", "W2_rates", "W2_rev"]
    }
    bs = {
        n: nc.dram_tensor(n, [D], F32, kind="ExternalInput")
        for n in ["b1_follows", "b1_rates", "b1_rev", "b2_follows", "b2_rates", "b2_rev"]
    }
    iota_in = nc.dram_tensor("iota512", [P, WIN], F32, kind="ExternalInput")
    ident_in = nc.dram_tensor("ident", [P, P], F32, kind="ExternalInput")
    streams = {}
    for r, sched in scheds.items():
        streams[r] = dict(
            gidx=nc.dram_tensor(f"gidx_{r}", [16, sched.Ttot * 8], I16, kind="ExternalInput"),
            colw=nc.dram_tensor(f"colw_{r}", [P, sched.Ttot], F16, kind="ExternalInput"),
            norm=nc.dram_tensor(f"norm_{r}", [P, sched.Ttot], F16, kind="ExternalInput"),
        )
    out_user = nc.dram_tensor("out_user", [SU, D], BF16, kind="ExternalOutput")
    out_item = nc.dram_tensor("out_item", [SI, D], BF16, kind="ExternalOutput")

    # library preamble: ensure Q7 mlp ucode (dma_gather) is loaded before
    # any tile-scheduled gather executes
    with nc.Block() as blk:
        @blk.gpsimd
        def _(gpsimd):
            gpsimd.load_library(mlp)

    with tile.TileContext(nc) as tc:
        with (
            tc.tile_pool(name="const", bufs=1) as cp,
            tc.tile_pool(name="gsl", bufs=10) as gp,
            tc.tile_pool(name="Sp", bufs=8) as sp,
            tc.tile_pool(name="swp", bufs=4) as swp,
            tc.tile_pool(name="gwp", bufs=4) as gwp,
            tc.tile_pool(name="agg", bufs=6) as aggp,
            tc.tile_pool(name="outp", bufs=6) as outp,
            tc.tile_pool(name="ps", bufs=4, space="PSUM") as pp,
            tc.tile_pool(name="psh", bufs=2, space="PSUM") as ph_pool,
            tc.tile_pool(name="pstr", bufs=2, space="PSUM") as ptr,
            tc.tile_pool(name="dram", bufs=1, space="DRAM") as dp,
        ):
            # ---- constants ----
            iota_t = cp.tile([P, WIN], F32, tag="iota")
            nc.sync.dma_start(iota_t[:], iota_in[:])
            ident_t = cp.tile([P, P], F32, tag="ident")
            nc.sync.dma_start(ident_t[:], ident_in[:])
            W_t = {}
            for n, Wd in Ws.items():
                W_t[n] = cp.tile([P, P], F32, tag=f"W_{n}", name=f"W_{n}")
                nc.sync.dma_start(W_t[n][:], Wd[:])
            b_t = {}
            for n, b in bs.items():
                b_t[n] = cp.tile([P, 1], F32, tag=f"b_{n}", name=f"bt_{n}")
                nc.sync.dma_start(b_t[n][:], b[:].unsqueeze(1))
            b1uv = cp.tile([P, 1], F32, tag="b1uv")
            nc.vector.tensor_tensor(
                out=b1uv[:], in0=b_t["b1_follows"][:], in1=b_t["b1_rev"][:],
                op=mybir.AluOpType.add,
            )
            nc.vector.tensor_scalar_mul(b1uv[:], b1uv[:], 0.5)
            b2uv = cp.tile([P, 1], F32, tag="b2uv")
            nc.vector.tensor_tensor(
                out=b2uv[:], in0=b_t["b2_follows"][:], in1=b_t["b2_rev"][:],
                op=mybir.AluOpType.add,
            )
            nc.vector.tensor_scalar_mul(b2uv[:], b2uv[:], 0.5)

            # ---- streams resident in SBUF (gidx streamed per window) ----
            st = {}
            for r, sched in scheds.items():
                colw = cp.tile([P, sched.Ttot], F16, tag=f"colw_{r}", name=f"colwt_{r}")
                nc.sync.dma_start(colw[:], streams[r]["colw"][:])
                nrm = cp.tile([P, sched.Ttot], F16, tag=f"norm_{r}", name=f"normt_{r}")
                nc.sync.dma_start(nrm[:], streams[r]["norm"][:])
                st[r] = dict(colw=colw, norm=nrm)

            # ---- DRAM tables ----
            # Shared = fast HBM-HBM collective path, but allows only one
            # writer -> use Local when REPEAT-timing re-runs the layers
            ag_space = "Shared" if os.environ.get("REPEAT", "1") == "1" else "Local"
            xu_f32 = dp.tile([SU, D], F32, tag="xu_f32")
            xi_f32 = dp.tile([SI, D], F32, tag="xi_f32")
            xu_full = dp.tile([N_U, D], F32, tag="xu_full", addr_space=ag_space)
            xi_full = dp.tile([N_I, D], F32, tag="xi_full", addr_space=ag_space)
            u_slice = dp.tile([SU, D], F32, tag="u_slice")
            it_slice = dp.tile([SI, D], F32, tag="it_slice")
            u_full = dp.tile([N_U, D], F32, tag="u_full", addr_space=ag_space)
            it_full = dp.tile([N_I, D], F32, tag="it_full", addr_space=ag_space)

            # upcast input shards (sync DMA + ACT copy: SWDGE queues are
            # reserved for gathers -- tile locks DMASW sem lanes per queue)
            def upcast(dst_ap, src, rows):
                for b in range(_cdiv(rows, P)):
                    r0, r1 = b * P, min((b + 1) * P, rows)
                    tb = outp.tile([P, P], BF16, tag="castb")
                    nc.sync.dma_start(tb[: r1 - r0, :], src[r0:r1, :])
                    tf = outp.tile([P, P], F32, tag="castf")
                    nc.scalar.activation(
                        out=tf[: r1 - r0, :], in_=tb[: r1 - r0, :],
                        func=mybir.ActivationFunctionType.Copy,
                    )
                    nc.sync.dma_start(dst_ap[r0:r1, :], tf[: r1 - r0, :])

            upcast(xu_f32[:], xu_in.ap(), SU)
            upcast(xi_f32[:], xi_in.ap(), SI)
            grp = [list(range(NCORES))]
            if not ABL_NOAG:
                nc.gpsimd.collective_compute(
                    "AllGather", mybir.AluOpType.bypass, replica_groups=grp,
                    ins=[xu_f32[:]], outs=[xu_full[:]],
                )
                nc.gpsimd.collective_compute(
                    "AllGather", mybir.AluOpType.bypass, replica_groups=grp,
                    ins=[xi_f32[:]], outs=[xi_full[:]],
                )

            qctr = [0]

            def agg_window(rel, w, table_ap):
                """PSUM tile [128 feat, WIN] = sum_e x[src_e] (x) one-hot."""
                sched = scheds[rel]
                ops = sched.ops[w]
                total = sum(g for (_, _, g) in ops)
                psum = pp.tile([P, WIN], F32, tag="aggps")
                if total == 0:
                    nc.vector.memset(psum[:], 0.0)
                    return psum
                # fp32 per-window casts of the fp16 colw/norm streams
                # (DVE is_equal requires an fp32 scalar operand)
                t_w0 = int(sched.t0[w, 0])
                T_win = int(sched.T[w].sum())
                colw32 = swp.tile([P, T_win], F32, tag="colw32")
                nc.scalar.activation(
                    out=colw32[:], in_=st[rel]["colw"][:, t_w0 : t_w0 + T_win],
                    func=mybir.ActivationFunctionType.Copy,
                )
                norm32 = swp.tile([P, T_win], F32, tag="norm32")
                nc.scalar.activation(
                    out=norm32[:], in_=st[rel]["norm"][:, t_w0 : t_w0 + T_win],
                    func=mybir.ActivationFunctionType.Copy,
                )
                # window slice of the wrapped gather indices, replicated
                # into all 8 partition groups via a broadcast AP
                gw = gwp.tile([P, T_win * 8], I16, tag="gw")
                gsrc = streams[rel]["gidx"][:, t_w0 * 8 : (t_w0 + T_win) * 8]
                nc.sync.dma_start(gw[:], gsrc.unsqueeze(0).broadcast_to([8, 16, T_win * 8]))
                done = 0
                for (c, t_start, g) in ops:
                    if ABL_NOGATHER:
                        slab = None
                    else:
                        slab = gp.tile([P, GMAX, P], F32, tag="gsl")
                        nidx = g * P
                        c0 = (t_start - t_w0) * 8
                        nc.gpsimd.dma_gather(
                            slab[:, :g, :],
                            table_ap[c * CHUNK : c * CHUNK + sched.chunk_rows[c], :],
                            gw[:, c0 : c0 + nidx // 16],
                            nidx,
                            nidx,
                            P,
                            queue_num=qctr[0] % NQUEUES,
                        )
                        qctr[0] += 1
                    for j in range(g):
                        t = t_start + j - t_w0
                        if ABL_NOS:
                            S = iota_t
                        else:
                            S = sp.tile([P, WIN], F32, tag="S")
                            nc.vector.tensor_scalar(
                                out=S[:],
                                in0=iota_t[:],
                                scalar1=colw32[:, t : t + 1],
                                scalar2=norm32[:, t : t + 1],
                                op0=mybir.AluOpType.is_equal,
                                op1=mybir.AluOpType.mult,
                            )
                        lhs = ident_t if ABL_NOGATHER else slab[:, j, :]
                        nc.tensor.matmul(
                            out=psum[:],
                            lhsT=lhs,
                            rhs=S[:],
                            start=(done == 0),
                            stop=(done == total - 1),
                        )
                        done += 1
                return psum

            def write_windows(h_sb, w, nrows, dst_ap, out_dt):
                for blk_i in range(_cdiv(nrows, P)):
                    r0, r1 = blk_i * P, min((blk_i + 1) * P, nrows)
                    ptile = ptr.tile([P, P], F32, tag="ptr")
                    nc.tensor.transpose(
                        out=ptile[: r1 - r0, :],
                        in_=h_sb[:, r0:r1],
                        identity=ident_t[:],
                    )
                    ob = outp.tile([P, P], out_dt, tag="ob")
                    nc.scalar.activation(
                        out=ob[: r1 - r0, :], in_=ptile[: r1 - r0, :],
                        func=mybir.ActivationFunctionType.Copy,
                    )
                    nc.sync.dma_start(
                        dst_ap[w * WIN + r0 : w * WIN + r1, :], ob[: r1 - r0, :]
                    )

            def user_layer(l, table_u, table_i, dst_ap, shard_rows, relu, out_dt):
                Wf = W_t[f"W{l}_follows"]
                Wv = W_t[f"W{l}_rev"]
                bias = b1uv if l == 1 else b2uv
                for w in range(_cdiv(shard_rows, WIN)):
                    nrows = min(WIN, shard_rows - w * WIN)
                    psF = agg_window("follows", w, table_u)
                    aggF = aggp.tile([P, WIN], F32, tag="aggF")
                    nc.scalar.activation(
                        out=aggF[:], in_=psF[:], func=mybir.ActivationFunctionType.Copy
                    )
                    psV = agg_window("rev", w, table_i)
                    aggV = aggp.tile([P, WIN], F32, tag="aggV")
                    nc.scalar.activation(
                        out=aggV[:], in_=psV[:], func=mybir.ActivationFunctionType.Copy
                    )
                    hps = ph_pool.tile([P, WIN], F32, tag="hps")
                    nc.tensor.matmul(out=hps[:], lhsT=Wf[:], rhs=aggF[:], start=True, stop=False)
                    nc.tensor.matmul(out=hps[:], lhsT=Wv[:], rhs=aggV[:], start=False, stop=True)
                    h_sb = aggp.tile([P, WIN], F32, tag="h_sb")
                    if relu:
                        nc.scalar.activation(
                            out=h_sb[:], in_=hps[:],
                            func=mybir.ActivationFunctionType.Relu,
                            bias=bias[:], scale=0.5,
                        )
                    else:
                        nc.vector.tensor_scalar(
                            out=h_sb[:], in0=hps[:],
                            scalar1=0.5, scalar2=bias[:],
                            op0=mybir.AluOpType.mult, op1=mybir.AluOpType.add,
                        )
                    write_windows(h_sb, w, nrows, dst_ap, out_dt)

            def item_layer(l, table_u, dst_ap, shard_rows, relu, out_dt):
                Wr = W_t[f"W{l}_rates"]
                bias = b_t[f"b{l}_rates"]
                for w in range(_cdiv(shard_rows, WIN)):
                    nrows = min(WIN, shard_rows - w * WIN)
                    psR = agg_window("rates", w, table_u)
                    aggR = aggp.tile([P, WIN], F32, tag="aggR")
                    nc.scalar.activation(
                        out=aggR[:], in_=psR[:], func=mybir.ActivationFunctionType.Copy
                    )
                    hps = ph_pool.tile([P, WIN], F32, tag="hps")
                    nc.tensor.matmul(out=hps[:], lhsT=Wr[:], rhs=aggR[:], start=True, stop=True)
                    h_sb = aggp.tile([P, WIN], F32, tag="h_sb")
                    if relu:
                        nc.scalar.activation(
                            out=h_sb[:], in_=hps[:],
                            func=mybir.ActivationFunctionType.Relu,
                            bias=bias[:], scale=1.0,
                        )
                    else:
                        nc.vector.tensor_scalar(
                            out=h_sb[:], in0=hps[:],
                            scalar1=1.0, scalar2=bias[:],
                            op0=mybir.AluOpType.mult, op1=mybir.AluOpType.add,
                        )
                    write_windows(h_sb, w, nrows, dst_ap, out_dt)

            # ---- layers (REPEAT>1 only for timing experiments) ----
            for _rep in range(int(os.environ.get("REPEAT", "1"))):
                user_layer(1, xu_full[:], xi_full[:], u_slice[:], SU, relu=True, out_dt=F32)
                if not ABL_NOAG:
                    nc.gpsimd.collective_compute(
                        "AllGather", mybir.AluOpType.bypass, replica_groups=grp,
                        ins=[u_slice[:]], outs=[u_full[:]],
                    )
                item_layer(1, xu_full[:], it_slice[:], SI, relu=True, out_dt=F32)
                if not ABL_NOAG:
                    nc.gpsimd.collective_compute(
                        "AllGather", mybir.AluOpType.bypass, replica_groups=grp,
                        ins=[it_slice[:]], outs=[it_full[:]],
                    )
                # layer 2 (rates first: only needs u_full)
                if not ABL_L1ONLY:
                    item_layer(2, u_full[:], out_item.ap(), SI, relu=False, out_dt=BF16)
                    user_layer(2, u_full[:], it_full[:], out_user.ap(), SU, relu=False, out_dt=BF16)

    nc.compile()
    return nc


def prepare(inputs):
    """Host-side prep + program build. Returns (nc, in_maps)."""
    cfg = dict(CFG)
    N_U = inputs["x_user"].shape[0]
    N_I = inputs["x_item"].shape[0]
    cfg.update(N_U=N_U, N_I=N_I, E=len(inputs["follows_src"]))
    SU, SI = N_U // NCORES, N_I // NCORES

    rel_edges = {
        "follows": (inputs["follows_src"], inputs["follows_dst"], N_U, N_U),
        "rates": (inputs["rates_src"], inputs["rates_dst"], N_U, N_I),
        "rev": (inputs["rev_src"], inputs["rev_dst"], N_I, N_U),
    }
    scheds, packs = {}, {}
    for r, (s, d, ns, nd) in rel_edges.items():
        sched, per_core = prep_relation(
            np.asarray(s).astype(np.int64), np.asarray(d).astype(np.int64), ns, nd
        )
        scheds[r] = sched
        packs[r] = per_core

    nc = build_program(cfg, scheds)

    iota512 = np.broadcast_to(np.arange(WIN, dtype=np.float32), (P, WIN)).copy()
    ident = np.eye(P, dtype=np.float32)
    xu = np.asarray(inputs["x_user"]).astype(ml_dtypes.bfloat16)
    xi = np.asarray(inputs["x_item"]).astype(ml_dtypes.bfloat16)
    common = {
        n: np.asarray(inputs[n])
        for n in [
            "W1_follows", "W1_rates", "W1_rev", "W2_follows", "W2_rates", "W2_rev",
            "b1_follows", "b1_rates", "b1_rev", "b2_follows", "b2_rates", "b2_rev",
        ]
    }
    in_maps = []
    for k in range(NCORES):
        m = dict(common, iota512=iota512, ident=ident)
        m["xu_shard"] = xu[k * SU : (k + 1) * SU]
        m["xi_shard"] = xi[k * SI : (k + 1) * SI]
        for r in rel_edges:
            gidx, colw, nrmw = packs[r][k]
            m[f"gidx_{r}"] = gidx
            m[f"colw_{r}"] = colw
            m[f"norm_{r}"] = nrmw
        in_maps.append(m)
    return nc, in_maps


def assemble(results):
    u2 = np.concatenate([results[k]["out_user"] for k in range(NCORES)], axis=0)
    i2 = np.concatenate([results[k]["out_item"] for k in range(NCORES)], axis=0)
    return np.concatenate([u2, i2], axis=0).astype(np.float32)


def kernel(**inputs):
    nc, in_maps = prepare(inputs)
    res = run_bass_kernel_spmd(nc, in_maps, list(range(NCORES)))
    return assemble(res.results)


if __name__ == "__main__":
    pass


# revision 18
# speedup vs baseline: 1.5748x; 1.0632x over previous
"""Trainium2 Bass kernel for the 2-layer heterogeneous GCN encoder.

Strategy (8 NeuronCores, SPMD):
  - Shard each relation's edges by dst-node owner: core k owns user rows
    [k*12500,(k+1)*12500) and item rows [k*6250,(k+1)*6250).
  - Aggregate raw features first (segment_sum(x[src]*norm, dst) per
    512-row dst window via a one-hot S matmul), transform per window
    afterwards: out = agg @ W + b.
  - Feature rows fetched with dma_gather (SWDGE custom op): up to 512
    rows per instruction, round-robined across 2 SWDGE queues (two Q7
    descriptor-gen cores run in parallel; measured ~3ns/row vs ~8.4ns
    single-queue and ~72x less instruction overhead than per-tile
    indirect DMA).
  - dma_gather takes int16 indices, so each relation's edges are grouped
    by (dst window, 32K src chunk); indices are chunk-relative.
  - x_user/x_item are uploaded bf16 and SHARDED (1/8 per core), cast to
    fp32 and AllGathered device-side into full HBM tables -- avoids
    uploading the full tables 8x.
  - Layer-1 outputs AllGathered fp32; layer 2 reads the gathered tables.
  - Outputs written bf16 (host casts back to fp32).

Self-contained: hardcodes problem shapes; host does only index-side prep
(degrees/norms from int32 edge lists, sharding, sorting, packing).
"""

import os
import sys

sys.path.insert(0, "/opt/trn_rl_repo")

import numpy as np
import ml_dtypes

import concourse.bass as bass
import concourse.bacc as bacc
import concourse.mybir as mybir
import concourse.tile as tile
from concourse.bass_utils import run_bass_kernel_spmd
from concourse.library_config import mlp

P = 128
WIN = 512  # dst rows per aggregation window (one PSUM bank)
CHUNK = 32768  # max rows addressable by one int16-indexed gather
GMAX = 4  # max tiles (of 128 rows) per dma_gather op -> <=512 rows
NCORES = 8
NQUEUES = 2
SCRATCH = 65536  # SWDGE ring: 4096 descriptors per queue per direction
F32 = mybir.dt.float32
F32R = mybir.dt.float32r  # relaxed fp32: 4x PE throughput at N>=256
F16 = mybir.dt.float16
BF16 = mybir.dt.bfloat16
I16 = mybir.dt.int16

CFG = dict(N_U=100000, N_I=50000, E=1600000, D=128)

# relation -> (src table, dst type)
RELS = {
    "follows": ("user", "user"),
    "rates": ("user", "item"),
    "rev": ("item", "user"),
}


def _cdiv(a, b):
    return (a + b - 1) // b


class RelSched:
    """Harmonized (across cores) tile schedule for one relation."""

    __slots__ = ("nwin", "nchunk", "T", "t0", "Ttot", "ops", "chunk_rows")

    def __init__(self, nwin, nchunk, T, chunk_rows):
        self.nwin, self.nchunk = nwin, nchunk
        self.T = T  # [nwin, nchunk] tiles per group
        self.chunk_rows = chunk_rows  # rows per chunk of the src table
        t0 = np.zeros((nwin, nchunk), np.int64)
        flat = T.reshape(-1)
        t0.reshape(-1)[1:] = np.cumsum(flat)[:-1]
        self.t0 = t0
        self.Ttot = int(flat.sum())
        # gather ops per window: list of (chunk, t_start, g_tiles)
        self.ops = []
        for w in range(nwin):
            lst = []
            for c in range(nchunk):
                t, rem = int(t0[w, c]), int(T[w, c])
                while rem > 0:
                    g = min(GMAX, rem)
                    lst.append((c, t, g))
                    t += g
                    rem -= g
            self.ops.append(lst)


def prep_relation(src, dst, n_src, n_dst, ncores=NCORES):
    """Shard edges by dst owner, group by (dst window, src chunk), pack
    per-core streams. Returns (RelSched, per-core (gidx, colw, norm))."""
    shard = n_dst // ncores
    nwin = _cdiv(shard, WIN)
    nchunk = _cdiv(n_src, CHUNK)
    chunk_rows = [min(CHUNK, n_src - c * CHUNK) for c in range(nchunk)]

    ones = np.ones_like(src, dtype=np.float64)
    deg_s = np.bincount(src, weights=ones, minlength=n_src)
    deg_d = np.bincount(dst, weights=ones, minlength=n_dst)
    inv_s = np.where(deg_s > 0, 1.0 / np.sqrt(deg_s), 0.0)
    inv_d = np.where(deg_d > 0, 1.0 / np.sqrt(deg_d), 0.0)
    norm = (inv_s[src] * inv_d[dst]).astype(np.float32)

    owner = dst // shard
    dloc = dst - owner * shard
    w = dloc // WIN
    col = dloc - w * WIN
    chunk = src // CHUNK
    srcrel = (src - chunk * CHUNK).astype(np.int64)

    key = (owner * nwin + w) * nchunk + chunk
    order = np.argsort(key, kind="stable")
    key_s = key[order]
    counts = np.bincount(key_s, minlength=ncores * nwin * nchunk).reshape(
        ncores, nwin, nchunk
    )
    T = _cdiv(counts.max(axis=0), P)  # [nwin, nchunk]
    sched = RelSched(nwin, nchunk, T, chunk_rows)

    # position of each (sorted) edge within its (owner,w,chunk) group
    grp_start = np.zeros(ncores * nwin * nchunk + 1, np.int64)
    grp_start[1:] = np.cumsum(counts.reshape(-1))
    j = np.arange(len(src)) - grp_start[key_s]
    # global token index within the core's stream
    t0_flat = sched.t0.reshape(-1)  # [nwin*nchunk]
    wc_key = key_s % (nwin * nchunk)
    g_tok = t0_flat[wc_key] * P + j

    src_o, col_o, norm_o, own_o = (
        srcrel[order],
        col[order],
        norm[order],
        owner[order],
    )
    per_core = []
    ntok = sched.Ttot * P
    for k in range(ncores):
        sel = own_o == k
        g_k = g_tok[sel]
        gidx = np.zeros((16, ntok // 16), np.int16)
        colw = np.zeros((P, sched.Ttot), np.float16)
        nrmw = np.zeros((P, sched.Ttot), np.float16)
        gidx[g_k % 16, g_k // 16] = src_o[sel].astype(np.int16)
        colw[g_k % P, g_k // P] = col_o[sel].astype(np.float16)
        nrmw[g_k % P, g_k // P] = norm_o[sel].astype(np.float16)
        per_core.append((gidx, colw, nrmw))
    return sched, per_core


def build_program(cfg, scheds):
    N_U, N_I, D = cfg["N_U"], cfg["N_I"], cfg["D"]
    SU, SI = N_U // NCORES, N_I // NCORES

    ABL_NOAG = os.environ.get("ABL_NOAG") == "1"
    ABL_L1ONLY = os.environ.get("ABL_L1ONLY") == "1"
    ABL_NOGATHER = os.environ.get("ABL_NOGATHER") == "1"
    ABL_NOS = os.environ.get("ABL_NOS") == "1"

    nc = bacc.Bacc(
        "TRN2",
        target_bir_lowering=False,
        dynamic_dma_scratch_size=SCRATCH,
        num_swdge_queues=NQUEUES,
    )

    xu_in = nc.dram_tensor("xu_shard", [SU, D], BF16, kind="ExternalInput")
    xi_in = nc.dram_tensor("xi_shard", [SI, D], BF16, kind="ExternalInput")
    Ws = {
        n: nc.dram_tensor(n, [D, D], F32, kind="ExternalInput")
        for n in ["W1_follows", "W1_rates", "W1_rev", "W2_follows", "W2_rates", "W2_rev"]
    }
    bs = {
        n: nc.dram_tensor(n, [D], F32, kind="ExternalInput")
        for n in ["b1_follows", "b1_rates", "b1_rev", "b2_follows", "b2_rates", "b2_rev"]
    }
    iota_in = nc.dram_tensor("iota512", [P, WIN], F32, kind="ExternalInput")
    ident_in = nc.dram_tensor("ident", [P, P], F32, kind="ExternalInput")
    streams = {}
    for r, sched in scheds.items():
        streams[r] = dict(
            gidx=nc.dram_tensor(f"gidx_{r}", [16, sched.Ttot * 8], I16, kind="ExternalInput"),
            colw=nc.dram_tensor(f"colw_{r}", [P, sched.Ttot], F16, kind="ExternalInput"),
            norm=nc.dram_tensor(f"norm_{r}", [P, sched.Ttot], F16, kind="ExternalInput"),
        )
    out_user = nc.dram_tensor("out_user", [SU, D], BF16, kind="ExternalOutput")
    out_item = nc.dram_tensor("out_item", [SI, D], BF16, kind="ExternalOutput")

    # library preamble: ensure Q7 mlp ucode (dma_gather) is loaded before
    # any tile-scheduled gather executes
    with nc.Block() as blk:
        @blk.gpsimd
        def _(gpsimd):
            gpsimd.load_library(mlp)

    with tile.TileContext(nc) as tc:
        with (
            tc.tile_pool(name="const", bufs=1) as cp,
            tc.tile_pool(name="gsl", bufs=10) as gp,
            tc.tile_pool(name="Sp", bufs=8) as sp,
            tc.tile_pool(name="swp", bufs=4) as swp,
            tc.tile_pool(name="gwp", bufs=4) as gwp,
            tc.tile_pool(name="agg", bufs=6) as aggp,
            tc.tile_pool(name="outp", bufs=6) as outp,
            tc.tile_pool(name="ps", bufs=4, space="PSUM") as pp,
            tc.tile_pool(name="psh", bufs=2, space="PSUM") as ph_pool,
            tc.tile_pool(name="pstr", bufs=2, space="PSUM") as ptr,
            tc.tile_pool(name="dram", bufs=1, space="DRAM") as dp,
        ):
            # ---- constants ----
            iota_t = cp.tile([P, WIN], F32, tag="iota")
            nc.sync.dma_start(iota_t[:], iota_in[:])
            ident_t = cp.tile([P, P], F32, tag="ident")
            nc.sync.dma_start(ident_t[:], ident_in[:])
            W_t = {}
            for n, Wd in Ws.items():
                wst = cp.tile([P, P], F32, tag=f"Wst_{n}", name=f"Wst_{n}")
                nc.sync.dma_start(wst[:], Wd[:])
                W_t[n] = cp.tile([P, P], F32R, tag=f"W_{n}", name=f"W_{n}")
                nc.scalar.activation(
                    out=W_t[n][:], in_=wst[:],
                    func=mybir.ActivationFunctionType.Copy,
                )
            b_t = {}
            for n, b in bs.items():
                b_t[n] = cp.tile([P, 1], F32, tag=f"b_{n}", name=f"bt_{n}")
                nc.sync.dma_start(b_t[n][:], b[:].unsqueeze(1))
            b1uv = cp.tile([P, 1], F32, tag="b1uv")
            nc.vector.tensor_tensor(
                out=b1uv[:], in0=b_t["b1_follows"][:], in1=b_t["b1_rev"][:],
                op=mybir.AluOpType.add,
            )
            nc.vector.tensor_scalar_mul(b1uv[:], b1uv[:], 0.5)
            b2uv = cp.tile([P, 1], F32, tag="b2uv")
            nc.vector.tensor_tensor(
                out=b2uv[:], in0=b_t["b2_follows"][:], in1=b_t["b2_rev"][:],
                op=mybir.AluOpType.add,
            )
            nc.vector.tensor_scalar_mul(b2uv[:], b2uv[:], 0.5)

            # ---- streams resident in SBUF (gidx streamed per window) ----
            st = {}
            for r, sched in scheds.items():
                colw = cp.tile([P, sched.Ttot], F16, tag=f"colw_{r}", name=f"colwt_{r}")
                nc.sync.dma_start(colw[:], streams[r]["colw"][:])
                nrm = cp.tile([P, sched.Ttot], F16, tag=f"norm_{r}", name=f"normt_{r}")
                nc.sync.dma_start(nrm[:], streams[r]["norm"][:])
                st[r] = dict(colw=colw, norm=nrm)

            # ---- DRAM tables ----
            # Shared = fast HBM-HBM collective path, but allows only one
            # writer -> use Local when REPEAT-timing re-runs the layers
            ag_space = "Shared" if os.environ.get("REPEAT", "1") == "1" else "Local"
            xu_f32 = dp.tile([SU, D], F32, tag="xu_f32")
            xi_f32 = dp.tile([SI, D], F32, tag="xi_f32")
            xu_full = dp.tile([N_U, D], F32, tag="xu_full", addr_space=ag_space)
            xi_full = dp.tile([N_I, D], F32, tag="xi_full", addr_space=ag_space)
            u_slice = dp.tile([SU, D], F32, tag="u_slice")
            it_slice = dp.tile([SI, D], F32, tag="it_slice")
            u_full = dp.tile([N_U, D], F32, tag="u_full", addr_space=ag_space)
            it_full = dp.tile([N_I, D], F32, tag="it_full", addr_space=ag_space)

            # upcast input shards (sync DMA + ACT copy: SWDGE queues are
            # reserved for gathers -- tile locks DMASW sem lanes per queue)
            def upcast(dst_ap, src, rows):
                for b in range(_cdiv(rows, P)):
                    r0, r1 = b * P, min((b + 1) * P, rows)
                    tb = outp.tile([P, P], BF16, tag="castb")
                    nc.sync.dma_start(tb[: r1 - r0, :], src[r0:r1, :])
                    tf = outp.tile([P, P], F32, tag="castf")
                    nc.scalar.activation(
                        out=tf[: r1 - r0, :], in_=tb[: r1 - r0, :],
                        func=mybir.ActivationFunctionType.Copy,
                    )
                    nc.sync.dma_start(dst_ap[r0:r1, :], tf[: r1 - r0, :])

            upcast(xu_f32[:], xu_in.ap(), SU)
            upcast(xi_f32[:], xi_in.ap(), SI)
            grp = [list(range(NCORES))]
            if not ABL_NOAG:
                nc.gpsimd.collective_compute(
                    "AllGather", mybir.AluOpType.bypass, replica_groups=grp,
                    ins=[xu_f32[:]], outs=[xu_full[:]],
                )
                nc.gpsimd.collective_compute(
                    "AllGather", mybir.AluOpType.bypass, replica_groups=grp,
                    ins=[xi_f32[:]], outs=[xi_full[:]],
                )

            qctr = [0]

            def agg_window(rel, w, table_ap):
                """PSUM tile [128 feat, WIN] = sum_e x[src_e] (x) one-hot."""
                sched = scheds[rel]
                ops = sched.ops[w]
                total = sum(g for (_, _, g) in ops)
                psum = pp.tile([P, WIN], F32, tag="aggps")
                if total == 0:
                    nc.vector.memset(psum[:], 0.0)
                    return psum
                # fp32 per-window casts of the fp16 colw/norm streams
                # (DVE is_equal requires an fp32 scalar operand)
                t_w0 = int(sched.t0[w, 0])
                T_win = int(sched.T[w].sum())
                colw32 = swp.tile([P, T_win], F32, tag="colw32")
                nc.scalar.activation(
                    out=colw32[:], in_=st[rel]["colw"][:, t_w0 : t_w0 + T_win],
                    func=mybir.ActivationFunctionType.Copy,
                )
                norm32 = swp.tile([P, T_win], F32, tag="norm32")
                nc.scalar.activation(
                    out=norm32[:], in_=st[rel]["norm"][:, t_w0 : t_w0 + T_win],
                    func=mybir.ActivationFunctionType.Copy,
                )
                # window slice of the wrapped gather indices, replicated
                # into all 8 partition groups via a broadcast AP
                gw = gwp.tile([P, T_win * 8], I16, tag="gw")
                gsrc = streams[rel]["gidx"][:, t_w0 * 8 : (t_w0 + T_win) * 8]
                nc.sync.dma_start(gw[:], gsrc.unsqueeze(0).broadcast_to([8, 16, T_win * 8]))
                done = 0
                for (c, t_start, g) in ops:
                    if ABL_NOGATHER:
                        slab = None
                    else:
                        slab = gp.tile([P, GMAX, P], F32R, tag="gsl")
                        nidx = g * P
                        c0 = (t_start - t_w0) * 8
                        nc.gpsimd.dma_gather(
                            slab[:, :g, :],
                            table_ap[c * CHUNK : c * CHUNK + sched.chunk_rows[c], :].bitcast(F32R),
                            gw[:, c0 : c0 + nidx // 16],
                            nidx,
                            nidx,
                            P,
                            queue_num=qctr[0] % NQUEUES,
                        )
                        qctr[0] += 1
                    for j in range(g):
                        t = t_start + j - t_w0
                        if ABL_NOS:
                            S = iota_t
                        else:
                            S = sp.tile([P, WIN], F32R, tag="S")
                            nc.vector.tensor_scalar(
                                out=S[:],
                                in0=iota_t[:],
                                scalar1=colw32[:, t : t + 1],
                                scalar2=norm32[:, t : t + 1],
                                op0=mybir.AluOpType.is_equal,
                                op1=mybir.AluOpType.mult,
                            )
                        lhs = ident_t if ABL_NOGATHER else slab[:, j, :]
                        nc.tensor.matmul(
                            out=psum[:],
                            lhsT=lhs if not ABL_NOGATHER else lhs.bitcast(F32R),
                            rhs=S[:],
                            start=(done == 0),
                            stop=(done == total - 1),
                        )
                        done += 1
                return psum

            def write_windows(h_sb, w, nrows, dst_ap, out_dt):
                for blk_i in range(_cdiv(nrows, P)):
                    r0, r1 = blk_i * P, min((blk_i + 1) * P, nrows)
                    ptile = ptr.tile([P, P], F32, tag="ptr")
                    nc.tensor.transpose(
                        out=ptile[: r1 - r0, :],
                        in_=h_sb[:, r0:r1],
                        identity=ident_t[:],
                    )
                    ob = outp.tile([P, P], out_dt, tag="ob")
                    nc.scalar.activation(
                        out=ob[: r1 - r0, :], in_=ptile[: r1 - r0, :],
                        func=mybir.ActivationFunctionType.Copy,
                    )
                    nc.sync.dma_start(
                        dst_ap[w * WIN + r0 : w * WIN + r1, :], ob[: r1 - r0, :]
                    )

            def user_layer(l, table_u, table_i, dst_ap, shard_rows, relu, out_dt):
                Wf = W_t[f"W{l}_follows"]
                Wv = W_t[f"W{l}_rev"]
                bias = b1uv if l == 1 else b2uv
                for w in range(_cdiv(shard_rows, WIN)):
                    nrows = min(WIN, shard_rows - w * WIN)
                    psF = agg_window("follows", w, table_u)
                    aggF = aggp.tile([P, WIN], F32R, tag="aggF")
                    nc.scalar.activation(
                        out=aggF[:], in_=psF[:], func=mybir.ActivationFunctionType.Copy
                    )
                    psV = agg_window("rev", w, table_i)
                    aggV = aggp.tile([P, WIN], F32R, tag="aggV")
                    nc.scalar.activation(
                        out=aggV[:], in_=psV[:], func=mybir.ActivationFunctionType.Copy
                    )
                    hps = ph_pool.tile([P, WIN], F32, tag="hps")
                    nc.tensor.matmul(out=hps[:], lhsT=Wf[:], rhs=aggF[:], start=True, stop=False)
                    nc.tensor.matmul(out=hps[:], lhsT=Wv[:], rhs=aggV[:], start=False, stop=True)
                    h_sb = aggp.tile([P, WIN], F32, tag="h_sb")
                    if relu:
                        nc.scalar.activation(
                            out=h_sb[:], in_=hps[:],
                            func=mybir.ActivationFunctionType.Relu,
                            bias=bias[:], scale=0.5,
                        )
                    else:
                        nc.vector.tensor_scalar(
                            out=h_sb[:], in0=hps[:],
                            scalar1=0.5, scalar2=bias[:],
                            op0=mybir.AluOpType.mult, op1=mybir.AluOpType.add,
                        )
                    write_windows(h_sb, w, nrows, dst_ap, out_dt)

            def item_layer(l, table_u, dst_ap, shard_rows, relu, out_dt):
                Wr = W_t[f"W{l}_rates"]
                bias = b_t[f"b{l}_rates"]
                for w in range(_cdiv(shard_rows, WIN)):
                    nrows = min(WIN, shard_rows - w * WIN)
                    psR = agg_window("rates", w, table_u)
                    aggR = aggp.tile([P, WIN], F32R, tag="aggR")
                    nc.scalar.activation(
                        out=aggR[:], in_=psR[:], func=mybir.ActivationFunctionType.Copy
                    )
                    hps = ph_pool.tile([P, WIN], F32, tag="hps")
                    nc.tensor.matmul(out=hps[:], lhsT=Wr[:], rhs=aggR[:], start=True, stop=True)
                    h_sb = aggp.tile([P, WIN], F32, tag="h_sb")
                    if relu:
                        nc.scalar.activation(
                            out=h_sb[:], in_=hps[:],
                            func=mybir.ActivationFunctionType.Relu,
                            bias=bias[:], scale=1.0,
                        )
                    else:
                        nc.vector.tensor_scalar(
                            out=h_sb[:], in0=hps[:],
                            scalar1=1.0, scalar2=bias[:],
                            op0=mybir.AluOpType.mult, op1=mybir.AluOpType.add,
                        )
                    write_windows(h_sb, w, nrows, dst_ap, out_dt)

            # ---- layers (REPEAT>1 only for timing experiments) ----
            for _rep in range(int(os.environ.get("REPEAT", "1"))):
                user_layer(1, xu_full[:], xi_full[:], u_slice[:], SU, relu=True, out_dt=F32)
                if not ABL_NOAG:
                    nc.gpsimd.collective_compute(
                        "AllGather", mybir.AluOpType.bypass, replica_groups=grp,
                        ins=[u_slice[:]], outs=[u_full[:]],
                    )
                item_layer(1, xu_full[:], it_slice[:], SI, relu=True, out_dt=F32)
                if not ABL_NOAG:
                    nc.gpsimd.collective_compute(
                        "AllGather", mybir.AluOpType.bypass, replica_groups=grp,
                        ins=[it_slice[:]], outs=[it_full[:]],
                    )
                # layer 2 (rates first: only needs u_full)
                if not ABL_L1ONLY:
                    item_layer(2, u_full[:], out_item.ap(), SI, relu=False, out_dt=BF16)
                    user_layer(2, u_full[:], it_full[:], out_user.ap(), SU, relu=False, out_dt=BF16)

    nc.compile()
    return nc


def prepare(inputs):
    """Host-side prep + program build. Returns (nc, in_maps)."""
    cfg = dict(CFG)
    N_U = inputs["x_user"].shape[0]
    N_I = inputs["x_item"].shape[0]
    cfg.update(N_U=N_U, N_I=N_I, E=len(inputs["follows_src"]))
    SU, SI = N_U // NCORES, N_I // NCORES

    rel_edges = {
        "follows": (inputs["follows_src"], inputs["follows_dst"], N_U, N_U),
        "rates": (inputs["rates_src"], inputs["rates_dst"], N_U, N_I),
        "rev": (inputs["rev_src"], inputs["rev_dst"], N_I, N_U),
    }
    scheds, packs = {}, {}
    for r, (s, d, ns, nd) in rel_edges.items():
        sched, per_core = prep_relation(
            np.asarray(s).astype(np.int64), np.asarray(d).astype(np.int64), ns, nd
        )
        scheds[r] = sched
        packs[r] = per_core

    nc = build_program(cfg, scheds)

    iota512 = np.broadcast_to(np.arange(WIN, dtype=np.float32), (P, WIN)).copy()
    ident = np.eye(P, dtype=np.float32)
    xu = np.asarray(inputs["x_user"]).astype(ml_dtypes.bfloat16)
    xi = np.asarray(inputs["x_item"]).astype(ml_dtypes.bfloat16)
    common = {
        n: np.asarray(inputs[n])
        for n in [
            "W1_follows", "W1_rates", "W1_rev", "W2_follows", "W2_rates", "W2_rev",
            "b1_follows", "b1_rates", "b1_rev", "b2_follows", "b2_rates", "b2_rev",
        ]
    }
    in_maps = []
    for k in range(NCORES):
        m = dict(common, iota512=iota512, ident=ident)
        m["xu_shard"] = xu[k * SU : (k + 1) * SU]
        m["xi_shard"] = xi[k * SI : (k + 1) * SI]
        for r in rel_edges:
            gidx, colw, nrmw = packs[r][k]
            m[f"gidx_{r}"] = gidx
            m[f"colw_{r}"] = colw
            m[f"norm_{r}"] = nrmw
        in_maps.append(m)
    return nc, in_maps


def assemble(results):
    u2 = np.concatenate([results[k]["out_user"] for k in range(NCORES)], axis=0)
    i2 = np.concatenate([results[k]["out_item"] for k in range(NCORES)], axis=0)
    return np.concatenate([u2, i2], axis=0).astype(np.float32)


def kernel(**inputs):
    nc, in_maps = prepare(inputs)
    res = run_bass_kernel_spmd(nc, in_maps, list(range(NCORES)))
    return assemble(res.results)


if __name__ == "__main__":
    pass


# revision 19
# speedup vs baseline: 4.1752x; 2.6513x over previous
"""Trainium2 Bass kernel for the 2-layer heterogeneous GCN encoder.

Strategy (8 NeuronCores, SPMD):
  - Shard each relation's edges by dst-node owner: core k owns user rows
    [k*12500,(k+1)*12500) and item rows [k*6250,(k+1)*6250).
  - Aggregate raw features first (segment_sum(x[src]*norm, dst) per
    512-row dst window via a one-hot S matmul), transform per window
    afterwards: out = agg @ W + b.
  - Feature rows fetched with dma_gather (SWDGE custom op): up to 512
    rows per instruction, round-robined across 2 SWDGE queues (two Q7
    descriptor-gen cores run in parallel; measured ~3ns/row vs ~8.4ns
    single-queue and ~72x less instruction overhead than per-tile
    indirect DMA).
  - dma_gather takes int16 indices, so each relation's edges are grouped
    by (dst window, 32K src chunk); indices are chunk-relative.
  - x_user/x_item are uploaded bf16 and SHARDED (1/8 per core), cast to
    fp32 and AllGathered device-side into full HBM tables -- avoids
    uploading the full tables 8x.
  - Layer-1 outputs AllGathered fp32; layer 2 reads the gathered tables.
  - Outputs written bf16 (host casts back to fp32).

Self-contained: hardcodes problem shapes; host does only index-side prep
(degrees/norms from int32 edge lists, sharding, sorting, packing).
"""

import os
import sys

sys.path.insert(0, "/opt/trn_rl_repo")

import numpy as np
import ml_dtypes

import concourse.bass as bass
import concourse.bacc as bacc
import concourse.mybir as mybir
import concourse.tile as tile
from concourse.bass_utils import run_bass_kernel_spmd
from concourse.library_config import mlp

P = 128
WIN = 512  # dst rows per aggregation window (one PSUM bank)
CHUNK = 32768  # max rows addressable by one int16-indexed gather
GMAX = 4  # max tiles (of 128 rows) per dma_gather op -> <=512 rows
NCORES = 8
NQUEUES = 2
SCRATCH = 65536  # SWDGE ring: 4096 descriptors per queue per direction
F32 = mybir.dt.float32
F32R = mybir.dt.float32r  # relaxed fp32: 4x PE throughput at N>=256
F16 = mybir.dt.float16
BF16 = mybir.dt.bfloat16
I16 = mybir.dt.int16

CFG = dict(N_U=100000, N_I=50000, E=1600000, D=128)

# relation -> (src table, dst type)
RELS = {
    "follows": ("user", "user"),
    "rates": ("user", "item"),
    "rev": ("item", "user"),
}


def _cdiv(a, b):
    return (a + b - 1) // b


class RelSched:
    """Harmonized (across cores) tile schedule for one relation."""

    __slots__ = ("nwin", "nchunk", "T", "t0", "Ttot", "ops", "chunk_rows")

    def __init__(self, nwin, nchunk, T, chunk_rows):
        self.nwin, self.nchunk = nwin, nchunk
        self.T = T  # [nwin, nchunk] tiles per group
        self.chunk_rows = chunk_rows  # rows per chunk of the src table
        t0 = np.zeros((nwin, nchunk), np.int64)
        flat = T.reshape(-1)
        t0.reshape(-1)[1:] = np.cumsum(flat)[:-1]
        self.t0 = t0
        self.Ttot = int(flat.sum())
        # gather ops per window: list of (chunk, t_start, g_tiles)
        self.ops = []
        for w in range(nwin):
            lst = []
            for c in range(nchunk):
                t, rem = int(t0[w, c]), int(T[w, c])
                while rem > 0:
                    g = min(GMAX, rem)
                    lst.append((c, t, g))
                    t += g
                    rem -= g
            self.ops.append(lst)


def prep_relation(src, dst, n_src, n_dst, ncores=NCORES):
    """Shard edges by dst owner, group by (dst window, src chunk), pack
    per-core streams. Returns (RelSched, per-core (gidx, colw, norm))."""
    shard = n_dst // ncores
    nwin = _cdiv(shard, WIN)
    nchunk = _cdiv(n_src, CHUNK)
    chunk_rows = [min(CHUNK, n_src - c * CHUNK) for c in range(nchunk)]

    ones = np.ones_like(src, dtype=np.float64)
    deg_s = np.bincount(src, weights=ones, minlength=n_src)
    deg_d = np.bincount(dst, weights=ones, minlength=n_dst)
    inv_s = np.where(deg_s > 0, 1.0 / np.sqrt(deg_s), 0.0)
    inv_d = np.where(deg_d > 0, 1.0 / np.sqrt(deg_d), 0.0)
    norm = (inv_s[src] * inv_d[dst]).astype(np.float32)

    owner = dst // shard
    dloc = dst - owner * shard
    w = dloc // WIN
    col = dloc - w * WIN
    chunk = src // CHUNK
    srcrel = (src - chunk * CHUNK).astype(np.int64)

    key = (owner * nwin + w) * nchunk + chunk
    order = np.argsort(key, kind="stable")
    key_s = key[order]
    counts = np.bincount(key_s, minlength=ncores * nwin * nchunk).reshape(
        ncores, nwin, nchunk
    )
    T = _cdiv(counts.max(axis=0), P)  # [nwin, nchunk]
    sched = RelSched(nwin, nchunk, T, chunk_rows)

    # position of each (sorted) edge within its (owner,w,chunk) group
    grp_start = np.zeros(ncores * nwin * nchunk + 1, np.int64)
    grp_start[1:] = np.cumsum(counts.reshape(-1))
    j = np.arange(len(src)) - grp_start[key_s]
    # global token index within the core's stream
    t0_flat = sched.t0.reshape(-1)  # [nwin*nchunk]
    wc_key = key_s % (nwin * nchunk)
    g_tok = t0_flat[wc_key] * P + j

    src_o, col_o, norm_o, own_o = (
        srcrel[order],
        col[order],
        norm[order],
        owner[order],
    )
    per_core = []
    ntok = sched.Ttot * P
    for k in range(ncores):
        sel = own_o == k
        g_k = g_tok[sel]
        gidx = np.zeros((16, ntok // 16), np.int16)
        colw = np.zeros((P, sched.Ttot), np.float16)
        nrmw = np.zeros((P, sched.Ttot), np.float16)
        gidx[g_k % 16, g_k // 16] = src_o[sel].astype(np.int16)
        colw[g_k % P, g_k // P] = col_o[sel].astype(np.float16)
        nrmw[g_k % P, g_k // P] = norm_o[sel].astype(np.float16)
        per_core.append((gidx, colw, nrmw))
    return sched, per_core


def build_program(cfg, scheds):
    N_U, N_I, D = cfg["N_U"], cfg["N_I"], cfg["D"]
    SU, SI = N_U // NCORES, N_I // NCORES

    ABL_NOAG = os.environ.get("ABL_NOAG") == "1"
    ABL_L1ONLY = os.environ.get("ABL_L1ONLY") == "1"
    ABL_NOGATHER = os.environ.get("ABL_NOGATHER") == "1"
    ABL_NOS = os.environ.get("ABL_NOS") == "1"

    nc = bacc.Bacc(
        "TRN2",
        target_bir_lowering=False,
        dynamic_dma_scratch_size=SCRATCH,
        num_swdge_queues=NQUEUES,
    )

    xu_in = nc.dram_tensor("xu_shard", [SU, D], BF16, kind="ExternalInput")
    xi_in = nc.dram_tensor("xi_shard", [SI, D], BF16, kind="ExternalInput")
    Ws = {
        n: nc.dram_tensor(n, [D, D], F32, kind="ExternalInput")
        for n in ["W1_follows", "W1_rates", "W1_rev", "W2_follows", "W2_rates", "W2_rev"]
    }
    bs = {
        n: nc.dram_tensor(n, [D], F32, kind="ExternalInput")
        for n in ["b1_follows", "b1_rates", "b1_rev", "b2_follows", "b2_rates", "b2_rev"]
    }
    iota_in = nc.dram_tensor("iota512", [P, WIN], F32, kind="ExternalInput")
    ident_in = nc.dram_tensor("ident", [P, P], F32, kind="ExternalInput")
    streams = {}
    for r, sched in scheds.items():
        streams[r] = dict(
            gidx=nc.dram_tensor(f"gidx_{r}", [16, sched.Ttot * 8], I16, kind="ExternalInput"),
            colw=nc.dram_tensor(f"colw_{r}", [P, sched.Ttot], F16, kind="ExternalInput"),
            norm=nc.dram_tensor(f"norm_{r}", [P, sched.Ttot], F16, kind="ExternalInput"),
        )
    out_user = nc.dram_tensor("out_user", [SU, D], BF16, kind="ExternalOutput")
    out_item = nc.dram_tensor("out_item", [SI, D], BF16, kind="ExternalOutput")

    # library preamble: ensure Q7 mlp ucode (dma_gather) is loaded before
    # any tile-scheduled gather executes
    with nc.Block() as blk:
        @blk.gpsimd
        def _(gpsimd):
            gpsimd.load_library(mlp)

    with tile.TileContext(nc) as tc:
        with (
            tc.tile_pool(name="const", bufs=1) as cp,
            tc.tile_pool(name="gsl", bufs=10) as gp,
            tc.tile_pool(name="Sp", bufs=8) as sp,
            tc.tile_pool(name="swp", bufs=4) as swp,
            tc.tile_pool(name="gwp", bufs=4) as gwp,
            tc.tile_pool(name="agg", bufs=6) as aggp,
            tc.tile_pool(name="outp", bufs=6) as outp,
            tc.tile_pool(name="ps", bufs=4, space="PSUM") as pp,
            tc.tile_pool(name="psh", bufs=2, space="PSUM") as ph_pool,
            tc.tile_pool(name="pstr", bufs=2, space="PSUM") as ptr,
            tc.tile_pool(name="dram", bufs=1, space="DRAM") as dp,
        ):
            # ---- constants ----
            iota_t = cp.tile([P, WIN], F32, tag="iota")
            nc.sync.dma_start(iota_t[:], iota_in[:])
            ident_st = cp.tile([P, P], F32, tag="ident_st")
            nc.sync.dma_start(ident_st[:], ident_in[:])
            ident_t = cp.tile([P, P], F32, tag="ident")
            nc.scalar.activation(
                out=ident_t[:], in_=ident_st[:],
                func=mybir.ActivationFunctionType.Copy,
            )
            ident_r = cp.tile([P, P], F32R, tag="ident_r")
            nc.scalar.activation(
                out=ident_r[:], in_=ident_st[:],
                func=mybir.ActivationFunctionType.Copy,
            )
            W_t = {}
            for n, Wd in Ws.items():
                wst = cp.tile([P, P], F32, tag=f"Wst_{n}", name=f"Wst_{n}")
                nc.sync.dma_start(wst[:], Wd[:])
                W_t[n] = cp.tile([P, P], F32R, tag=f"W_{n}", name=f"W_{n}")
                nc.scalar.activation(
                    out=W_t[n][:], in_=wst[:],
                    func=mybir.ActivationFunctionType.Copy,
                )
            b_t = {}
            for n, b in bs.items():
                b_t[n] = cp.tile([P, 1], F32, tag=f"b_{n}", name=f"bt_{n}")
                nc.sync.dma_start(b_t[n][:], b[:].unsqueeze(1))
            b1uv = cp.tile([P, 1], F32, tag="b1uv")
            nc.vector.tensor_tensor(
                out=b1uv[:], in0=b_t["b1_follows"][:], in1=b_t["b1_rev"][:],
                op=mybir.AluOpType.add,
            )
            nc.vector.tensor_scalar_mul(b1uv[:], b1uv[:], 0.5)
            b2uv = cp.tile([P, 1], F32, tag="b2uv")
            nc.vector.tensor_tensor(
                out=b2uv[:], in0=b_t["b2_follows"][:], in1=b_t["b2_rev"][:],
                op=mybir.AluOpType.add,
            )
            nc.vector.tensor_scalar_mul(b2uv[:], b2uv[:], 0.5)

            # ---- streams resident in SBUF (gidx streamed per window) ----
            st = {}
            for r, sched in scheds.items():
                colw = cp.tile([P, sched.Ttot], F16, tag=f"colw_{r}", name=f"colwt_{r}")
                nc.sync.dma_start(colw[:], streams[r]["colw"][:])
                nrm = cp.tile([P, sched.Ttot], F16, tag=f"norm_{r}", name=f"normt_{r}")
                nc.sync.dma_start(nrm[:], streams[r]["norm"][:])
                st[r] = dict(colw=colw, norm=nrm)

            # ---- DRAM tables ----
            # Shared = fast HBM-HBM collective path, but allows only one
            # writer -> use Local when REPEAT-timing re-runs the layers
            ag_space = "Shared" if os.environ.get("REPEAT", "1") == "1" else "Local"
            xu_f32 = dp.tile([SU, D], F32, tag="xu_f32")
            xi_f32 = dp.tile([SI, D], F32, tag="xi_f32")
            xu_full = dp.tile([N_U, D], F32, tag="xu_full", addr_space=ag_space)
            xi_full = dp.tile([N_I, D], F32, tag="xi_full", addr_space=ag_space)
            u_slice = dp.tile([SU, D], F32, tag="u_slice")
            it_slice = dp.tile([SI, D], F32, tag="it_slice")
            u_full = dp.tile([N_U, D], F32, tag="u_full", addr_space=ag_space)
            it_full = dp.tile([N_I, D], F32, tag="it_full", addr_space=ag_space)

            # upcast input shards (sync DMA + ACT copy: SWDGE queues are
            # reserved for gathers -- tile locks DMASW sem lanes per queue)
            def upcast(dst_ap, src, rows):
                for b in range(_cdiv(rows, P)):
                    r0, r1 = b * P, min((b + 1) * P, rows)
                    tb = outp.tile([P, P], BF16, tag="castb")
                    nc.sync.dma_start(tb[: r1 - r0, :], src[r0:r1, :])
                    tf = outp.tile([P, P], F32, tag="castf")
                    nc.scalar.activation(
                        out=tf[: r1 - r0, :], in_=tb[: r1 - r0, :],
                        func=mybir.ActivationFunctionType.Copy,
                    )
                    nc.sync.dma_start(dst_ap[r0:r1, :], tf[: r1 - r0, :])

            upcast(xu_f32[:], xu_in.ap(), SU)
            upcast(xi_f32[:], xi_in.ap(), SI)
            grp = [list(range(NCORES))]
            if not ABL_NOAG:
                nc.gpsimd.collective_compute(
                    "AllGather", mybir.AluOpType.bypass, replica_groups=grp,
                    ins=[xu_f32[:]], outs=[xu_full[:]],
                )
                nc.gpsimd.collective_compute(
                    "AllGather", mybir.AluOpType.bypass, replica_groups=grp,
                    ins=[xi_f32[:]], outs=[xi_full[:]],
                )

            qctr = [0]

            def agg_window(rel, w, table_ap):
                """PSUM tile [128 feat, WIN] = sum_e x[src_e] (x) one-hot."""
                sched = scheds[rel]
                ops = sched.ops[w]
                total = sum(g for (_, _, g) in ops)
                psum = pp.tile([P, WIN], F32, tag="aggps")
                if total == 0:
                    nc.vector.memset(psum[:], 0.0)
                    return psum
                # fp32 per-window casts of the fp16 colw/norm streams
                # (DVE is_equal requires an fp32 scalar operand)
                t_w0 = int(sched.t0[w, 0])
                T_win = int(sched.T[w].sum())
                colw32 = swp.tile([P, T_win], F32, tag="colw32")
                nc.scalar.activation(
                    out=colw32[:], in_=st[rel]["colw"][:, t_w0 : t_w0 + T_win],
                    func=mybir.ActivationFunctionType.Copy,
                )
                norm32 = swp.tile([P, T_win], F32, tag="norm32")
                nc.scalar.activation(
                    out=norm32[:], in_=st[rel]["norm"][:, t_w0 : t_w0 + T_win],
                    func=mybir.ActivationFunctionType.Copy,
                )
                # window slice of the wrapped gather indices, replicated
                # into all 8 partition groups via a broadcast AP
                gw = gwp.tile([P, T_win * 8], I16, tag="gw")
                gsrc = streams[rel]["gidx"][:, t_w0 * 8 : (t_w0 + T_win) * 8]
                nc.sync.dma_start(gw[:], gsrc.unsqueeze(0).broadcast_to([8, 16, T_win * 8]))
                done = 0
                for (c, t_start, g) in ops:
                    if ABL_NOGATHER:
                        slab = None
                    else:
                        slab = gp.tile([P, GMAX, P], F32R, tag="gsl")
                        nidx = g * P
                        c0 = (t_start - t_w0) * 8
                        nc.gpsimd.dma_gather(
                            slab[:, :g, :],
                            table_ap[c * CHUNK : c * CHUNK + sched.chunk_rows[c], :].bitcast(F32R),
                            gw[:, c0 : c0 + nidx // 16],
                            nidx,
                            nidx,
                            P,
                            queue_num=qctr[0] % NQUEUES,
                        )
                        qctr[0] += 1
                    for j in range(g):
                        t = t_start + j - t_w0
                        if ABL_NOS:
                            S = iota_t
                        else:
                            S = sp.tile([P, WIN], F32R, tag="S")
                            nc.vector.tensor_scalar(
                                out=S[:],
                                in0=iota_t[:],
                                scalar1=colw32[:, t : t + 1],
                                scalar2=norm32[:, t : t + 1],
                                op0=mybir.AluOpType.is_equal,
                                op1=mybir.AluOpType.mult,
                            )
                        lhs = ident_r if ABL_NOGATHER else slab[:, j, :]
                        nc.tensor.matmul(
                            out=psum[:],
                            lhsT=lhs,
                            rhs=S[:],
                            start=(done == 0),
                            stop=(done == total - 1),
                        )
                        done += 1
                return psum

            def write_windows(h_sb, w, nrows, dst_ap, out_dt):
                for blk_i in range(_cdiv(nrows, P)):
                    r0, r1 = blk_i * P, min((blk_i + 1) * P, nrows)
                    ptile = ptr.tile([P, P], F32, tag="ptr")
                    nc.tensor.transpose(
                        out=ptile[: r1 - r0, :],
                        in_=h_sb[:, r0:r1],
                        identity=ident_t[:],
                    )
                    ob = outp.tile([P, P], out_dt, tag="ob")
                    nc.scalar.activation(
                        out=ob[: r1 - r0, :], in_=ptile[: r1 - r0, :],
                        func=mybir.ActivationFunctionType.Copy,
                    )
                    nc.sync.dma_start(
                        dst_ap[w * WIN + r0 : w * WIN + r1, :], ob[: r1 - r0, :]
                    )

            def user_layer(l, table_u, table_i, dst_ap, shard_rows, relu, out_dt):
                Wf = W_t[f"W{l}_follows"]
                Wv = W_t[f"W{l}_rev"]
                bias = b1uv if l == 1 else b2uv
                for w in range(_cdiv(shard_rows, WIN)):
                    nrows = min(WIN, shard_rows - w * WIN)
                    psF = agg_window("follows", w, table_u)
                    aggF = aggp.tile([P, WIN], F32R, tag="aggF")
                    nc.scalar.activation(
                        out=aggF[:], in_=psF[:], func=mybir.ActivationFunctionType.Copy
                    )
                    psV = agg_window("rev", w, table_i)
                    aggV = aggp.tile([P, WIN], F32R, tag="aggV")
                    nc.scalar.activation(
                        out=aggV[:], in_=psV[:], func=mybir.ActivationFunctionType.Copy
                    )
                    hps = ph_pool.tile([P, WIN], F32, tag="hps")
                    nc.tensor.matmul(out=hps[:], lhsT=Wf[:], rhs=aggF[:], start=True, stop=False)
                    nc.tensor.matmul(out=hps[:], lhsT=Wv[:], rhs=aggV[:], start=False, stop=True)
                    h_sb = aggp.tile([P, WIN], F32, tag="h_sb")
                    if relu:
                        nc.scalar.activation(
                            out=h_sb[:], in_=hps[:],
                            func=mybir.ActivationFunctionType.Relu,
                            bias=bias[:], scale=0.5,
                        )
                    else:
                        nc.vector.tensor_scalar(
                            out=h_sb[:], in0=hps[:],
                            scalar1=0.5, scalar2=bias[:],
                            op0=mybir.AluOpType.mult, op1=mybir.AluOpType.add,
                        )
                    write_windows(h_sb, w, nrows, dst_ap, out_dt)

            def item_layer(l, table_u, dst_ap, shard_rows, relu, out_dt):
                Wr = W_t[f"W{l}_rates"]
                bias = b_t[f"b{l}_rates"]
                for w in range(_cdiv(shard_rows, WIN)):
                    nrows = min(WIN, shard_rows - w * WIN)
                    psR = agg_window("rates", w, table_u)
                    aggR = aggp.tile([P, WIN], F32R, tag="aggR")
                    nc.scalar.activation(
                        out=aggR[:], in_=psR[:], func=mybir.ActivationFunctionType.Copy
                    )
                    hps = ph_pool.tile([P, WIN], F32, tag="hps")
                    nc.tensor.matmul(out=hps[:], lhsT=Wr[:], rhs=aggR[:], start=True, stop=True)
                    h_sb = aggp.tile([P, WIN], F32, tag="h_sb")
                    if relu:
                        nc.scalar.activation(
                            out=h_sb[:], in_=hps[:],
                            func=mybir.ActivationFunctionType.Relu,
                            bias=bias[:], scale=1.0,
                        )
                    else:
                        nc.vector.tensor_scalar(
                            out=h_sb[:], in0=hps[:],
                            scalar1=1.0, scalar2=bias[:],
                            op0=mybir.AluOpType.mult, op1=mybir.AluOpType.add,
                        )
                    write_windows(h_sb, w, nrows, dst_ap, out_dt)

            # ---- layers (REPEAT>1 only for timing experiments) ----
            for _rep in range(int(os.environ.get("REPEAT", "1"))):
                user_layer(1, xu_full[:], xi_full[:], u_slice[:], SU, relu=True, out_dt=F32)
                if not ABL_NOAG:
                    nc.gpsimd.collective_compute(
                        "AllGather", mybir.AluOpType.bypass, replica_groups=grp,
                        ins=[u_slice[:]], outs=[u_full[:]],
                    )
                item_layer(1, xu_full[:], it_slice[:], SI, relu=True, out_dt=F32)
                if not ABL_NOAG:
                    nc.gpsimd.collective_compute(
                        "AllGather", mybir.AluOpType.bypass, replica_groups=grp,
                        ins=[it_slice[:]], outs=[it_full[:]],
                    )
                # layer 2 (rates first: only needs u_full)
                if not ABL_L1ONLY:
                    item_layer(2, u_full[:], out_item.ap(), SI, relu=False, out_dt=BF16)
                    user_layer(2, u_full[:], it_full[:], out_user.ap(), SU, relu=False, out_dt=BF16)

    nc.compile()
    return nc


def prepare(inputs):
    """Host-side prep + program build. Returns (nc, in_maps)."""
    cfg = dict(CFG)
    N_U = inputs["x_user"].shape[0]
    N_I = inputs["x_item"].shape[0]
    cfg.update(N_U=N_U, N_I=N_I, E=len(inputs["follows_src"]))
    SU, SI = N_U // NCORES, N_I // NCORES

    rel_edges = {
        "follows": (inputs["follows_src"], inputs["follows_dst"], N_U, N_U),
        "rates": (inputs["rates_src"], inputs["rates_dst"], N_U, N_I),
        "rev": (inputs["rev_src"], inputs["rev_dst"], N_I, N_U),
    }
    scheds, packs = {}, {}
    for r, (s, d, ns, nd) in rel_edges.items():
        sched, per_core = prep_relation(
            np.asarray(s).astype(np.int64), np.asarray(d).astype(np.int64), ns, nd
        )
        scheds[r] = sched
        packs[r] = per_core

    nc = build_program(cfg, scheds)

    iota512 = np.broadcast_to(np.arange(WIN, dtype=np.float32), (P, WIN)).copy()
    ident = np.eye(P, dtype=np.float32)
    xu = np.asarray(inputs["x_user"]).astype(ml_dtypes.bfloat16)
    xi = np.asarray(inputs["x_item"]).astype(ml_dtypes.bfloat16)
    common = {
        n: np.asarray(inputs[n])
        for n in [
            "W1_follows", "W1_rates", "W1_rev", "W2_follows", "W2_rates", "W2_rev",
            "b1_follows", "b1_rates", "b1_rev", "b2_follows", "b2_rates", "b2_rev",
        ]
    }
    in_maps = []
    for k in range(NCORES):
        m = dict(common, iota512=iota512, ident=ident)
        m["xu_shard"] = xu[k * SU : (k + 1) * SU]
        m["xi_shard"] = xi[k * SI : (k + 1) * SI]
        for r in rel_edges:
            gidx, colw, nrmw = packs[r][k]
            m[f"gidx_{r}"] = gidx
            m[f"colw_{r}"] = colw
            m[f"norm_{r}"] = nrmw
        in_maps.append(m)
    return nc, in_maps


def assemble(results):
    u2 = np.concatenate([results[k]["out_user"] for k in range(NCORES)], axis=0)
    i2 = np.concatenate([results[k]["out_item"] for k in range(NCORES)], axis=0)
    return np.concatenate([u2, i2], axis=0).astype(np.float32)


def kernel(**inputs):
    nc, in_maps = prepare(inputs)
    res = run_bass_kernel_spmd(nc, in_maps, list(range(NCORES)))
    return assemble(res.results)


if __name__ == "__main__":
    pass
